# revision 13
# baseline (speedup 1.0000x reference)
"""Trainium2 Bass kernel for the CMPO2/GTN MPS-chain contraction.

Computation (see harness reference): for each sample s,
    v0  = psi_first^T x[s,0]                                  [D]
    v_{i+1}[e] = sum_{d,p} v_i[d] psi_mid[i][d,e,p] x[s,1+i,p]   (62 steps)
    out_vec[s] = sum_{d,p} v[d] psi_last[d,p,:] x[s,63,p]     [O]
    out[s] = c * out_vec[s]   with c the (batch-independent) phi-chain scalar.

Strategy: data-parallel over batch across 8 cores (1024 samples/core),
MPS parameters replicated.  Per middle step the contraction is mapped as
    u[s,(p,d)] = v[s,d] * x[s,p]        (outer product, fp16, p-major rows)
    v_new      = u @ A_flat             (PE matmul, K=2048 in 16 chunks of 128)
The per-sample v broadcast is free: the A stationaries are column-duplicated
so each accumulation chain outputs [vT; vT] on all 128 PSUM partitions, and
the DVE outer-product ops read the evacuated [vT; vT] tile through a
stride-0 (broadcast_to) access pattern, so no duplicate copies of v are
materialized.  The x-side partition broadcast (input data, not dependent on
v) is split: 12 of the 16 K-chunks are precomputed on the host and streamed
from HBM (fp16, DMA-bound side), the remaining 4 chunks are produced on-chip
by one-hot PE matmuls from a small [8, BL] x-slice and evacuated to SBUF by
the scalar engine — balancing DMA (HBM roofline) against PE and ACT, which
have headroom.  fp16 overflow is prevented by folding static power-of-2
scales (derived from a host-side subsample) into the A weights; the inverse
scale is folded into the phi-chain scalar, computed on-device in fp32.
"""

import numpy as np

N_CORES = 8
B, Q, P, D, L, O = 8192, 64, 32, 64, 64, 10
BL = B // N_CORES          # batch per core
TN = 512                   # matmul free-dim tile (one PSUM bank of fp32)
NT = BL // TN              # N tiles per batch shard
NCH = (D * P) // 128       # 16 K-chunks of 128 over (p,d)
NMID = L - 2               # 62 middle sites
NSITE = NMID + 1           # sites with an outer-product step (1..63)
NBC = 4                    # chunks produced on-chip by one-hot PE matmuls
NDMA = NCH - NBC           # chunks streamed from HBM (12)
SH_LAST = 6                # 2^SH_LAST folded into psi_last (fp16 subnormal avoidance)
VBAND = 16.0               # target |v| band for the scale schedule

# global row r in 0..2047 of u/(A rows): p = r//64 ; d = r%64
_P_IDX = np.repeat(np.arange(P), D)          # [2048]
_D_IDX = np.tile(np.arange(D), P)            # [2048]

_cached = {}


def _ensure_path():
    import sys
    for p in ("/opt/trn_rl_repo", "/root/.axon_site/_ro/trn_rl_repo"):
        try:
            import concourse  # noqa: F401
            return
        except Exception:
            if p not in sys.path:
                sys.path.insert(0, p)
    import concourse  # noqa: F401


def _build_program():
    """Build + compile the Bass/Tile program (shared by all 8 cores)."""
    _ensure_path()
    from concourse import bacc, tile, mybir

    dt = mybir.dt
    nc = bacc.Bacc(
        "TRN2",
        target_bir_lowering=False,
        debug=False,
        enable_asserts=False,
        num_devices=N_CORES,
    )

    a_d = nc.dram_tensor("a_w", [NMID, 128, NCH * D], dt.float16, kind="ExternalInput").ap()
    xba_d = nc.dram_tensor("xba", [NSITE, 128, 8 * BL], dt.float16, kind="ExternalInput").ap()
    xbb_d = nc.dram_tensor("xbb", [NSITE, 128, NDMA % 8 * BL], dt.float16, kind="ExternalInput").ap()
    xs_d = nc.dram_tensor("xs", [(NSITE + 3) // 4, 2 * NBC, 4 * BL], dt.float16, kind="ExternalInput").ap()
    oh_d = nc.dram_tensor("oh", [2 * NBC, NBC * 128], dt.float16, kind="ExternalInput").ap()
    x0_d = nc.dram_tensor("x0", [P, BL], dt.float16, kind="ExternalInput").ap()
    pf_d = nc.dram_tensor("pf", [P, 128], dt.float16, kind="ExternalInput").ap()
    pl_d = nc.dram_tensor("pl", [128, NCH * O], dt.float16, kind="ExternalInput").ap()
    pm_d = nc.dram_tensor("phim", [D, NMID * D], dt.float32, kind="ExternalInput").ap()
    w0_d = nc.dram_tensor("w0", [D, 1], dt.float32, kind="ExternalInput").ap()
    plc_d = nc.dram_tensor("phil", [D, 1], dt.float32, kind="ExternalInput").ap()
    out_d = nc.dram_tensor("out", [O, BL], dt.float32, kind="ExternalOutput").ap()

    with tile.TileContext(nc) as tc:
        with tc.tile_pool(name="const", bufs=1) as cpool, \
             tc.tile_pool(name="aw", bufs=2) as apool, \
             tc.tile_pool(name="xbp", bufs=6) as xbpool, \
             tc.tile_pool(name="awd", bufs=2) as adpool, \
             tc.tile_pool(name="xsp", bufs=2) as xspool, \
             tc.tile_pool(name="vrp", bufs=2) as vrpool, \
             tc.tile_pool(name="up", bufs=3) as upool, \
             tc.tile_pool(name="misc", bufs=1) as mpool, \
             tc.tile_pool(name="wp", bufs=2) as wpool, \
             tc.tile_pool(name="pvp", bufs=4, space="PSUM") as pvpool, \
             tc.tile_pool(name="pbp", bufs=1, space="PSUM") as pbpool, \
             tc.tile_pool(name="phpp", bufs=1, space="PSUM") as phpool:

            # --- constants / per-core inputs resident in SBUF ---
            pf_sb = cpool.tile([P, 128], dt.float16, name="pf_sb")
            nc.sync.dma_start(out=pf_sb, in_=pf_d)
            pl_sb = cpool.tile([128, NCH * O], dt.float16, name="pl_sb")
            nc.sync.dma_start(out=pl_sb, in_=pl_d)
            pm_sb = cpool.tile([D, NMID * D], dt.float32, name="pm_sb")
            nc.sync.dma_start(out=pm_sb, in_=pm_d)
            plc_sb = cpool.tile([D, 1], dt.float32, name="plc_sb")
            nc.sync.dma_start(out=plc_sb, in_=plc_d)
            x0_sb = cpool.tile([P, BL], dt.float16, name="x0_sb")
            nc.sync.dma_start(out=x0_sb, in_=x0_d)
            oh_sb = cpool.tile([2 * NBC, NBC * 128], dt.float16, name="oh_sb")
            nc.sync.dma_start(out=oh_sb, in_=oh_d)

            w_cur = wpool.tile([D, 1], dt.float32, name="wv")
            nc.sync.dma_start(out=w_cur, in_=w0_d)

            # --- v0 = [psi_first^T | psi_first^T] @ x0 -> [v0 ; v0] ---
            pv_cur = []
            for t in range(NT):
                pv = pvpool.tile([128, TN], dt.float32, name="pv")
                nc.tensor.matmul(out=pv, lhsT=pf_sb,
                                 rhs=x0_sb[:, t * TN:(t + 1) * TN],
                                 start=True, stop=True)
                pv_cur.append(pv)

            xs_cur = [None]

            def stage_x(i):
                """Issue DMAs + one-hot PE broadcast producing step i's xb
                tiles.  Called one step ahead of consumption so the bcast
                matmuls/evacuations fill engine-idle time instead of
                blocking the step's critical path.  The small x-slices for
                the one-hot matmuls are fetched four sites per transfer."""
                xq_a = xbpool.tile([128, 8 * BL], dt.float16, name="xq_a", tag="xq")
                nc.sync.dma_start(out=xq_a, in_=xba_d[i])
                xq_b = xbpool.tile([128, 8 * BL], dt.float16, name="xq_b", tag="xq")
                nc.scalar.dma_start(out=xq_b[:, :(NDMA - 8) * BL], in_=xbb_d[i])
                if i % 4 == 0:
                    xs_sb = xspool.tile([2 * NBC, 4 * BL], dt.float16, name="xs_sb")
                    nc.sync.dma_start(out=xs_sb, in_=xs_d[i // 4])
                    xs_cur[0] = xs_sb
                xoff = (i % 4) * BL
                for c in range(NBC):
                    pb = pbpool.tile([128, 2 * TN], dt.float32, name="pb")
                    for t in range(NT):
                        nc.tensor.matmul(
                            out=pb[:, t * TN:(t + 1) * TN],
                            lhsT=oh_sb[:, c * 128:(c + 1) * 128],
                            rhs=xs_cur[0][:, xoff + t * TN:xoff + (t + 1) * TN],
                            start=True, stop=True)
                    nc.scalar.copy(
                        out=xq_b[:, (4 + c) * BL:(5 + c) * BL], in_=pb)
                return xq_a, xq_b

            po = None
            cur_x = stage_x(0)
            for i in range(NSITE):
                last = (i == NMID)
                xq_a, xq_b = cur_x
                if not last:
                    cur_x = stage_x(i + 1)
                # evacuate [vT; vT] into v2 per N-tile half so each half's
                # outer products and matmuls can proceed while the other
                # half is still in flight.
                v2 = vrpool.tile([128, BL], dt.float16, name="v2")
                for t in range(NT):
                    nc.scalar.copy(out=v2[:, t * TN:(t + 1) * TN], in_=pv_cur[t])

                if not last:
                    # stream the un-duplicated A chunk-columns; duplicate
                    # on-chip with ONE scalar-engine copy whose input reads
                    # each chunk twice through a stride-0 (broadcast_to) AP
                    # (innermost stays dense so the copy runs in 4x mode).
                    a_raw = apool.tile([128, NCH * D], dt.float16, name="a_raw")
                    nc.scalar.dma_start(out=a_raw, in_=a_d[i])
                    a_sb = adpool.tile([128, NCH * 128], dt.float16, name="a_sb")
                    nc.scalar.copy(
                        out=a_sb.rearrange("p (c j e) -> p c j e", c=NCH, j=2, e=D),
                        in_=a_raw.rearrange("p (c o e) -> p c o e", c=NCH, o=1)
                                 .broadcast_to([128, NCH, 2, D]))
                    pv_nxt = []
                    for t in range(NT):
                        pv = pvpool.tile([128, TN], dt.float32, name="pv")
                        pv_nxt.append(pv)
                else:
                    po = []
                    for t in range(NT):
                        p_o = pvpool.tile([O, TN], dt.float32, name="pv")
                        po.append(p_o)

                # outer products and matmuls, emitted per N-tile half so the
                # two halves software-pipeline.  One DVE op per (half, xb
                # tile): out/in1 are [128, 8, TN] strided views, in0 is the
                # v2 half broadcast along the 8-chunk dim with a stride-0
                # access pattern (stays in 2x_1P mode: innermost is dense).
                us = [upool.tile([128, 8 * BL], dt.float16, name="u2", tag="us")
                      for _ in range(2)]
                out_ps = po if last else pv_nxt
                for t in range(NT):
                    sl = slice(t * TN, (t + 1) * TN)
                    v2b = v2[:, sl].rearrange("p (o s) -> p o s", o=1) \
                                   .broadcast_to([128, 8, TN])
                    for g, xq in enumerate((xq_a, xq_b)):
                        nc.vector.tensor_mul(
                            us[g].rearrange("p (b s) -> p b s", b=8)[:, :, sl],
                            v2b,
                            xq.rearrange("p (b s) -> p b s", b=8)[:, :, sl])
                    for c in range(NCH):
                        g, b = c // 8, c % 8
                        if last:
                            lhs = pl_sb[:, c * O:(c + 1) * O]
                        else:
                            lhs = a_sb[:, c * 128:(c + 1) * 128]
                        nc.tensor.matmul(
                            out=out_ps[t],
                            lhsT=lhs,
                            rhs=us[g][:, b * BL + t * TN:b * BL + (t + 1) * TN],
                            start=(c == 0), stop=(c == NCH - 1))
                if not last:
                    # phi chain matvec, interleaved (PE fp32, tiny)
                    php = phpool.tile([D, 1], dt.float32, name="php")
                    nc.tensor.matmul(out=php, lhsT=pm_sb[:, i * D:(i + 1) * D],
                                     rhs=w_cur, start=True, stop=True)
                    w_nxt = wpool.tile([D, 1], dt.float32, name="wv")
                    nc.scalar.copy(out=w_nxt, in_=php)
                    w_cur = w_nxt
                    pv_cur = pv_nxt

            # --- c = w^T phi_last' ; broadcast to O partitions; scale output ---
            cps = phpool.tile([1, 1], dt.float32, name="php")
            nc.tensor.matmul(out=cps, lhsT=plc_sb, rhs=w_cur, start=True, stop=True)
            c_sb = mpool.tile([1, 1], dt.float32, name="c_sb")
            nc.scalar.copy(out=c_sb, in_=cps)
            c10_sb = mpool.tile([O, 1], dt.float32, name="c10_sb")
            nc.gpsimd.partition_broadcast(c10_sb, c_sb)

            out_sb = mpool.tile([O, BL], dt.float32, name="out_sb")
            for t in range(NT):
                nc.scalar.mul(out=out_sb[:, t * TN:(t + 1) * TN], in_=po[t], mul=c10_sb)
            nc.sync.dma_start(out=out_d, in_=out_sb)

    nc.compile()
    return nc


def _scale_schedule(x, psi_first, psi_mid, nsub=128):
    """Static per-step power-of-2 downscales keeping |v| in a small band."""
    xs = np.asarray(x[:nsub], np.float32)
    v = xs[:, 0] @ np.asarray(psi_first, np.float32).T
    ks = []
    for i in range(NMID):
        A = np.asarray(psi_mid[i], np.float32)            # [d, e, p]
        xi = xs[:, 1 + i]                                  # [s, p]
        u = np.einsum('sd,sp->sdp', v, xi).reshape(nsub, D * P)
        v = u @ A.transpose(0, 2, 1).reshape(D * P, D)
        vm = float(np.abs(v).max())
        k = 0
        while vm * 2.0 ** (-k) > VBAND:
            k += 1
        ks.append(k)
        v = v * 2.0 ** (-k)
    return ks


def kernel(x, psi_first, psi_mid, psi_last, phi_first, phi_mid, phi_last):
    _ensure_path()
    from concourse import bass_utils

    f16 = np.float16
    x = np.asarray(x, np.float32)
    psi_first = np.asarray(psi_first, np.float32)
    psi_mid = np.asarray(psi_mid, np.float32)
    psi_last = np.asarray(psi_last, np.float32)
    phi_first = np.asarray(phi_first, np.float32)
    phi_mid = np.asarray(phi_mid, np.float32)
    phi_last = np.asarray(phi_last, np.float32)

    if "nc" not in _cached:
        _cached["nc"] = _build_program()
    nc = _cached["nc"]

    ks = _scale_schedule(x, psi_first, psi_mid)

    # --- shared weight-side arrays (p-major rows, duplicated columns) ---
    scales = (2.0 ** -np.asarray(ks, np.float64)).astype(np.float32)
    # A2[i, r, e] = psi_mid[i, d(r), e, p(r)] * s_i  -> [62, 2048, 64]
    A2 = psi_mid.transpose(0, 1, 3, 2)[:, _D_IDX, _P_IDX, :]        # [62, 2048, 64]
    A2 = A2 * scales[:, None, None]
    A2c = A2.reshape(NMID, NCH, 128, D)
    a_host = np.ascontiguousarray(
        A2c.transpose(0, 2, 1, 3).reshape(NMID, 128, NCH * D)
    ).astype(f16)

    pf_host = np.concatenate([psi_first.T, psi_first.T], axis=1).astype(f16)  # [32, 128]

    # pl2[r, o] = psi_last[d(r), p(r), o] * 2^SH -> chunked [128, 16*O]
    pl2 = (psi_last * (2.0 ** SH_LAST))[_D_IDX, _P_IDX, :]          # [2048, O]
    pl_host = np.ascontiguousarray(
        pl2.reshape(NCH, 128, O).transpose(1, 0, 2).reshape(128, NCH * O)
    ).astype(f16)

    phiM = phi_mid[np.arange(NMID), :, :, np.arange(1, NMID + 1)]   # [62, e, f]
    pm_host = np.ascontiguousarray(phiM.transpose(1, 0, 2).reshape(D, NMID * D)).astype(np.float32)
    w0_host = np.ascontiguousarray(phi_first[:, 0:1]).astype(np.float32)
    plc_host = np.ascontiguousarray(
        phi_last[:, Q - 1:Q] * (2.0 ** (sum(ks) - SH_LAST))
    ).astype(np.float32)

    # one-hot stationaries for the on-chip broadcast of chunks 12..15:
    # S[p', c*128 + r] = 1 where 24 + p' == 2*(12+c) + r//64
    oh_host = np.zeros((2 * NBC, NBC * 128), np.float32)
    for c in range(NBC):
        for r in range(128):
            pp = 2 * (NDMA + c) + r // 64 - 2 * NDMA
            oh_host[pp, c * 128 + r] = 1.0
    oh_host = oh_host.astype(f16)

    # --- per-core batch shards ---
    xt = x.transpose(1, 2, 0).astype(f16)         # [Q, P, B]
    x0_all = xt[0]                                # [P, B]
    in_maps = []
    for ci in range(N_CORES):
        sl = slice(ci * BL, (ci + 1) * BL)
        xsh = np.ascontiguousarray(xt[1:, :, sl])           # [63, P, BL]
        xb = xsh[:, _P_IDX, :]                              # [63, 2048, BL]
        # chunk c rows 128: tile A = chunks 0..7 as column blocks, tile B
        # holds only the DMA chunks 8..11 (12..15 are generated on-chip).
        xbc = xb.reshape(NSITE, NCH, 128, BL)
        xba = np.ascontiguousarray(
            xbc[:, :8].transpose(0, 2, 1, 3).reshape(NSITE, 128, 8 * BL))
        xbb = np.ascontiguousarray(
            xbc[:, 8:NDMA].transpose(0, 2, 1, 3).reshape(NSITE, 128, (NDMA - 8) * BL))
        nsg = (NSITE + 3) // 4
        xs_pad = np.zeros((4 * nsg, 2 * NBC, BL), f16)
        xs_pad[:NSITE] = xsh[:, 2 * NDMA:, :]
        xs_host = np.ascontiguousarray(
            xs_pad.reshape(nsg, 4, 2 * NBC, BL).transpose(0, 2, 1, 3)
                  .reshape(nsg, 2 * NBC, 4 * BL))
        in_maps.append({
            "a_w": a_host,
            "xba": xba,
            "xbb": xbb,
            "xs": xs_host,
            "oh": oh_host,
            "x0": np.ascontiguousarray(x0_all[:, sl]),
            "pf": pf_host,
            "pl": pl_host,
            "phim": pm_host,
            "w0": w0_host,
            "phil": plc_host,
        })

    res = bass_utils.run_bass_kernel_spmd(nc, in_maps, core_ids=list(range(N_CORES)))
    _cached["in_maps"] = in_maps

    out = np.empty((B, O), np.float32)
    for ci in range(N_CORES):
        out[ci * BL:(ci + 1) * BL, :] = res.results[ci]["out"].T
    return out
